# revision 13
# baseline (speedup 1.0000x reference)
"""Expert-parallel MoE (top-2 of 16 experts) for Trainium2, 8 NeuronCores.

Sharding strategy (per spec sharding_hint): expert-parallel. The 16 experts'
gate/up/down weights are sharded 2-per-core across the 8 cores. The router
(a [T,16] matmul + top-2, ~0.01% of total FLOPs) runs on the host at input-
shard time; the "all-to-all token dispatch" is realized as the host-side
gather that builds each core's token batch, and the top-2 weighted combine
is the host-side scatter-add at unshard time.

On-device per core (all heavy FLOPs + memory traffic):
  for each of its 2 experts, with X_e^T [H, C] (tokens on the PE free dim):
    G1 = gate_w[:, :2048]^T-tiles @ X^T      (PSUM f32)
    G2 = silu(gate_w[:, 2048:] @ X^T)        (ScalarE silu from PSUM)
    HH = G2 * G1 * (up_w @ X^T)              (VectorE, fp16)
    Y^T = down_w-tiles @ HH                  (PSUM f32 -> SBUF fp16 -> HBM)

All matmuls in fp16 (same PE rate as bf16, ~4x lower rounding error for
this data), f32 accumulation. Activations are laid out transposed
([H, C], tokens on the moving/free dim) so every weight matrix is used in
its natural [K, M] layout with zero on-device transposes.

Weight layout: the host pre-permutes each gate/up half into
[p=128, slab, itile, ktile, 128] so each (slab, itile) weight block is one
contiguous-per-partition 2KB DMA. DMAs are issued itile-major in exactly
the order the PE consumes them, which removes the DMA-starved stutter at
the start of the kernel (paid on EVERY rep of the timing loop, since For_i
puts an all-engine barrier between reps).
"""

import os

import numpy as np

import concourse.tile as tile
import concourse.mybir as mybir
from concourse import bacc
from concourse import bass_utils

N_CORES = 8
E = 16
H = 1024
I_G = 4096  # gate projection width
I_H = 2048  # up/down inner width
KB_H = H // 128  # 8 k-tiles for H-contraction
KB_I = I_H // 128  # 16 k-tiles for I_H-contraction

# 16-bit matmul dtype: fp16 and bf16 run at the same PE rate (1 cyc/row);
# fp16's 10 mantissa bits give ~4x lower rounding error for this data
# (all values well inside fp16 range).
F16 = mybir.dt.float16
F32 = mybir.dt.float32
NP_F16 = np.float16


def _ceil_mult(n: int, m: int) -> int:
    return ((n + m - 1) // m) * m


def _split_c(C: int, head: bool = False):
    """Split capacity C (multiple of 128) into PE free-dim tiles.

    Tiles are kept in {512, 384, 256} where possible: <=512 fits one PSUM
    bank in f32; >=256 keeps the per-tile LDWEIGHTS (~107ns) hidden under
    the matmul stream. With head=True the leading 512 is split into two
    256s so the first PSUM group only needs a 256-wide x chunk (halves the
    first DMA the PE waits on after the iteration barrier).
    Returns list of (offset, width)."""
    assert C % 128 == 0 and C > 0
    if C <= 512:
        widths = [C]
    else:
        q, r = divmod(C, 512)
        if r == 0:
            widths = [512] * q
        elif r == 256:
            widths = [512] * q + [256]
        elif r == 384:
            widths = [512] * q + [384]
        else:  # r == 128
            widths = [512] * (q - 1) + [384, 256]
    if head and widths[0] == 512:
        widths = [256, 256] + widths[1:]
    out = []
    off = 0
    for w in widths:
        out.append((off, w))
        off += w
    assert off == C
    return out


def _chunk2(seq):
    return [seq[i : i + 2] for i in range(0, len(seq), 2)]


# warm_head/warm_tail: number of 512-wide sacrificial matmuls on a scratch
# tile at the start/end of each rep. They execute inside PE-idle windows
# (head DMA wait, tail PSUM-drain) and keep the PE's HAM activity monitor
# from re-throttling the clock to 1.2 GHz across the For_i rep barrier.
# bulk: engine for non-critical (prefetch) DMAs - "gpsimd" (SWDGE ring,
# parallel to HWDGE) or "sync" (HWDGE).
# head_ct: split the first expert's leading 512 chunk into 256+256.
_OPT = dict(warm_head=12, warm_tail=9, bulk="gpsimd", head_ct=True)


def _bulk(nc):
    return nc.gpsimd if _OPT["bulk"] == "gpsimd" else nc.sync


def _expert_ffn(nc, wp, xp, hp, yp, tp, pp, xd, fw, dd, yd, C, first=False):
    """Emit one expert's FFN: y[H, C] = down( silu(g2)*g1*up ) for x[H, C].

    xd: DRAM [128, KB_H, C] f16 (p, k, c)
    fw: DRAM [128, 6, 8, KB_H, 128] f16 (p, slab, itile, k, ic);
        slab order (g2lo, g1lo, ulo, g2hi, g1hi, uhi)
    dd: DRAM [128, KB_I, H] f16 (p, k, hcols)
    yd: DRAM [H, C] f16 (viewed (hb p) c -> p hb c for writes)
    """
    ct = _split_c(C)
    ccg = _chunk2(ct)
    # head chunking (leading 512 -> 256+256) only for the very first psum
    # groups after the iteration barrier (il=0 of half 0): earliest PE
    # start on a half-size x chunk, without paying the extra instruction
    # issue cost across the whole expert.
    head = bool(first and _OPT["head_ct"])
    ccg0 = _chunk2(_split_c(C, head=True)) if head else ccg
    yr = yd.rearrange("(hb p) c -> p hb c", p=128)  # [128, 8, C]

    xs = xp.tile([128, KB_H, C], F16, tag="xt", name="xs")
    if not first:
        # bulk prefetch on the SWDGE (gpsimd) ring; two pieces to stay
        # under the descriptor-ring carveout
        for a in range(0, KB_H, 4):
            _bulk(nc).dma_start(out=xs[:, a : a + 4], in_=xd[:, a : a + 4])

    hh = hp.tile([128, KB_I, C], F16, tag="hh", name="hh")

    # ---- gate + up fused phase ----
    for half in range(2):
        sg2 = wp.tile([128, 8, KB_H, 128], F16, tag="w", name="sg2")
        sg1 = wp.tile([128, 8, KB_H, 128], F16, tag="w", name="sg1")
        su = wp.tile([128, 8, KB_H, 128], F16, tag="w", name="su")
        slabs = (sg2, sg1, su)
        if first and half == 0:
            # Critical path of the first PE work after the iteration
            # barrier: the HWDGE ring issues one DMA per ~625ns FIFO, so
            # put exactly the blocks the first PSUM group needs first, in
            # consumption order, and push everything else to the SWDGE
            # (gpsimd) ring which generates descriptors in parallel.
            off0, w0 = ccg0[0][0]
            nc.sync.dma_start(out=sg2[:, 0], in_=fw[:, half * 3 + 0, 0])
            nc.sync.dma_start(
                out=xs[:, :, off0 : off0 + w0], in_=xd[:, :, off0 : off0 + w0]
            )
            nc.sync.dma_start(out=sg1[:, 0], in_=fw[:, half * 3 + 1, 0])
            nc.sync.dma_start(out=su[:, 0], in_=fw[:, half * 3 + 2, 0])
            # rest of x (needed from the second psum group on)
            rem = C - w0
            step = _ceil_mult(rem // 2, 128) if rem > 512 else rem
            for a in range(w0, C, step):
                b = min(C, a + step)
                _bulk(nc).dma_start(out=xs[:, :, a:b], in_=xd[:, :, a:b])
            for it in range(1, 8):
                for s3 in range(3):
                    _bulk(nc).dma_start(
                        out=slabs[s3][:, it], in_=fw[:, half * 3 + s3, it]
                    )
        else:
            # itile-major, in consumption order, on the SWDGE ring
            for it in range(8):
                for s3 in range(3):
                    _bulk(nc).dma_start(
                        out=slabs[s3][:, it], in_=fw[:, half * 3 + s3, it]
                    )

        for il in range(8):
            i = half * 8 + il
            for cc in (ccg0 if (half == 0 and il == 0) else ccg):
                # --- g2 stream (silu half) ---
                pg2 = [pp.tile([128, w], F32, tag="ps", name="pg2") for (_, w) in cc]
                for k in range(KB_H):
                    for j, (off, w) in enumerate(cc):
                        nc.tensor.matmul(
                            pg2[j],
                            sg2[:, il, k, :],
                            xs[:, k, off : off + w],
                            start=(k == 0),
                            stop=(k == KB_H - 1),
                        )
                sil = []
                for j, (off, w) in enumerate(cc):
                    t = tp.tile([128, 512], F16, tag="t", name="t")
                    nc.scalar.activation(
                        out=t[:, :w],
                        in_=pg2[j],
                        func=mybir.ActivationFunctionType.Silu,
                    )
                    sil.append(t)
                # --- g1 stream ---
                pg1 = [pp.tile([128, w], F32, tag="ps", name="pg1") for (_, w) in cc]
                for k in range(KB_H):
                    for j, (off, w) in enumerate(cc):
                        nc.tensor.matmul(
                            pg1[j],
                            sg1[:, il, k, :],
                            xs[:, k, off : off + w],
                            start=(k == 0),
                            stop=(k == KB_H - 1),
                        )
                g12 = []
                for j, (off, w) in enumerate(cc):
                    t2 = tp.tile([128, 512], F16, tag="g12", name="t2")
                    nc.vector.tensor_mul(t2[:, :w], sil[j][:, :w], pg1[j])
                    g12.append(t2)
                # --- up stream ---
                pu = [pp.tile([128, w], F32, tag="ps", name="pu") for (_, w) in cc]
                for k in range(KB_H):
                    for j, (off, w) in enumerate(cc):
                        nc.tensor.matmul(
                            pu[j],
                            su[:, il, k, :],
                            xs[:, k, off : off + w],
                            start=(k == 0),
                            stop=(k == KB_H - 1),
                        )
                for j, (off, w) in enumerate(cc):
                    nc.vector.tensor_mul(
                        hh[:, i, off : off + w], g12[j][:, :w], pu[j]
                    )

    # ---- down phase ----
    dlo = wp.tile([128, KB_H, 1024], F16, tag="w", name="dlo")
    dhi = wp.tile([128, KB_H, 1024], F16, tag="w", name="dhi")
    for a in range(0, KB_H, 4):
        _bulk(nc).dma_start(out=dlo[:, a : a + 4, :], in_=dd[:, a : a + 4, :])
    for a in range(0, KB_H, 4):
        _bulk(nc).dma_start(
            out=dhi[:, a : a + 4, :], in_=dd[:, 8 + a : 8 + a + 4, :]
        )

    for h in range(8):
        ms = slice(h * 128, (h + 1) * 128)
        yl = yp.tile([128, C], F16, tag="y", name="yl")
        for cc in ccg:
            pd = [pp.tile([128, w], F32, tag="ps", name="pd") for (_, w) in cc]
            for k in range(KB_I):
                sl = dlo if k < 8 else dhi
                for j, (off, w) in enumerate(cc):
                    nc.tensor.matmul(
                        pd[j],
                        sl[:, k % 8, ms],
                        hh[:, k, off : off + w],
                        start=(k == 0),
                        stop=(k == KB_I - 1),
                    )
            for j, (off, w) in enumerate(cc):
                nc.vector.tensor_copy(yl[:, off : off + w], pd[j])
                nc.sync.dma_start(
                    out=yr[:, h, off : off + w], in_=yl[:, off : off + w]
                )


def _build_nc(CA: int, CB: int, reps: int = 1):
    """Build + compile the 2-expert-slot SPMD program (same on all cores).

    Slot b (the smaller capacity) runs first; slot a last, so the final
    PSUM drain + y writeback is its narrow trailing chunk.
    """
    nc = bacc.Bacc(
        "TRN2", target_bir_lowering=False, debug=False, num_devices=N_CORES
    )
    dram = {}
    for s, C in (("a", CA), ("b", CB)):
        dram[f"x{s}"] = nc.dram_tensor(
            f"x{s}", [128, KB_H, C], F16, kind="ExternalInput"
        ).ap()
        dram[f"fw{s}"] = nc.dram_tensor(
            f"fw{s}", [128, 6, 8, KB_H, 128], F16, kind="ExternalInput"
        ).ap()
        dram[f"d{s}"] = nc.dram_tensor(
            f"d{s}", [128, KB_I, H], F16, kind="ExternalInput"
        ).ap()
        dram[f"y{s}"] = nc.dram_tensor(
            f"y{s}", [H, C], F16, kind="ExternalOutput"
        ).ap()

    with tile.TileContext(nc) as tc:
        with (
            tc.tile_pool(name="wp", bufs=6) as wp,  # 16KB/part weight slabs
            tc.tile_pool(name="xp", bufs=2) as xp,
            tc.tile_pool(name="hp", bufs=1) as hp,
            tc.tile_pool(name="yp", bufs=3) as yp,
            tc.tile_pool(name="tp", bufs=4) as tp,
            tc.tile_pool(name="zp", bufs=1) as zp,
            tc.tile_pool(name="pp", bufs=8, space="PSUM") as pp,
        ):

            def _warm(z, n, nm):
                if not n:
                    return
                pz = pp.tile([128, 512], F32, tag="ps", name=nm)
                for _ in range(n):
                    nc.tensor.matmul(pz, z[:, :128], z, start=True, stop=True)

            def body():
                z = None
                if _OPT["warm_head"] or _OPT["warm_tail"]:
                    z = zp.tile([128, 512], F16, tag="z", name="z")
                    nc.vector.memset(z, 0.0)
                    _warm(z, _OPT["warm_head"], "pzh")
                for si, (s, C) in enumerate((("b", CB), ("a", CA))):
                    _expert_ffn(
                        nc,
                        wp,
                        xp,
                        hp,
                        yp,
                        tp,
                        pp,
                        dram[f"x{s}"],
                        dram[f"fw{s}"],
                        dram[f"d{s}"],
                        dram[f"y{s}"],
                        C,
                        first=(si == 0),
                    )
                if z is not None:
                    _warm(z, _OPT["warm_tail"], "pzt")

            if reps == 1:
                body()
            else:
                ET = mybir.EngineType
                with tc.For_i(
                    0,
                    reps,
                    1,
                    hint_engines=(ET.PE, ET.DVE, ET.Activation, ET.SP, ET.Pool),
                ):
                    body()

    nc.compile()
    return nc


_NC_CACHE: dict = {}


def _get_nc(CA: int, CB: int):
    key = (CA, CB)
    if key not in _NC_CACHE:
        _NC_CACHE[key] = _build_nc(CA, CB)
    return _NC_CACHE[key]


def _pack_fw(gw_e, uw_e):
    """[H, I_G] gate + [H, I_H] up -> [128, 6, 8, KB_H, 128] f16 slab tensor.

    slab s, itile it, ktile k, col ic holds W[k*128+p, base(s) + it*128+ic]
    with slab order (g2lo, g1lo, ulo, g2hi, g1hi, uhi).
    """
    parts = []
    for half in range(2):
        lo = half * 1024
        for src, base in ((gw_e, 2048 + lo), (gw_e, lo), (uw_e, lo)):
            blk = src[:, base : base + 1024]
            parts.append(
                blk.reshape(KB_H, 128, 8, 128).transpose(1, 2, 0, 3)
            )  # [p, it, k, ic]
    return np.stack(parts, axis=1).astype(NP_F16)  # [128, 6, 8, 8, 128]


def _route_and_shard(hs, rw, gw, uw, dw):
    """Host-side router + expert-parallel sharding of the full inputs."""
    B, S, _ = hs.shape
    T = B * S
    x = np.ascontiguousarray(hs.reshape(T, H)).astype(np.float32, copy=False)

    logits = x @ rw.astype(np.float32)  # [T, E]
    order = np.argsort(-logits, axis=1, kind="stable")[:, :2]
    l12 = np.take_along_axis(logits, order, axis=1).astype(np.float64)
    w1 = 1.0 / (1.0 + np.exp(l12[:, 1] - l12[:, 0]))  # renormalized top-2
    w2 = 1.0 - w1
    i1, i2 = order[:, 0], order[:, 1]

    idx, wts = [], []
    for e in range(E):
        m1 = i1 == e
        ide = np.nonzero(m1 | (i2 == e))[0]
        we = np.where(m1[ide], w1[ide], w2[ide]).astype(np.float32)
        idx.append(ide)
        wts.append(we)
    counts = np.array([len(v) for v in idx])

    # pair largest with smallest expert per core for load balance
    desc = np.argsort(-counts, kind="stable")
    slotA = [int(desc[c]) for c in range(N_CORES)]
    slotB = [int(desc[2 * N_CORES - 1 - c]) for c in range(N_CORES)]
    CA = max(256, _ceil_mult(int(counts[slotA].max()), 128))
    CB = max(256, _ceil_mult(int(counts[slotB].max()), 128))

    xT = np.ascontiguousarray(x.T).astype(NP_F16)  # [H, T]

    in_maps = []
    for c in range(N_CORES):
        m = {}
        for s, e, C in (("a", slotA[c], CA), ("b", slotB[c], CB)):
            xe = np.zeros((H, C), NP_F16)
            n = counts[e]
            xe[:, :n] = xT[:, idx[e]]
            m[f"x{s}"] = np.ascontiguousarray(
                xe.reshape(KB_H, 128, C).transpose(1, 0, 2)
            )
            m[f"fw{s}"] = _pack_fw(gw[e], uw[e])
            m[f"d{s}"] = np.ascontiguousarray(
                dw[e].reshape(KB_I, 128, H).transpose(1, 0, 2)
            ).astype(NP_F16)
        in_maps.append(m)

    meta = dict(
        B=B, S=S, T=T, idx=idx, wts=wts, counts=counts,
        slotA=slotA, slotB=slotB, CA=CA, CB=CB,
    )
    return in_maps, meta


def _combine(results, meta):
    """Host-side top-2 weighted combine (unshard)."""
    T = meta["T"]
    out = np.zeros((T, H), np.float32)
    for c in range(N_CORES):
        for s, e in (("a", meta["slotA"][c]), ("b", meta["slotB"][c])):
            n = int(meta["counts"][e])
            if n == 0:
                continue
            y = results[c][f"y{s}"][:, :n].astype(np.float32)  # [H, n]
            out[meta["idx"][e]] += meta["wts"][e][:, None] * y.T
    return out.reshape(meta["B"], meta["S"], H)


def _run_spmd(nc, in_maps):
    try:
        return bass_utils.run_bass_kernel_spmd(
            nc, in_maps, core_ids=list(range(N_CORES))
        )
    except ModuleNotFoundError:
        # axon NTFF profiling hook unavailable in this container; retry
        # with tracing force-disabled.
        os.environ["BASS_NEVER_TRACE"] = "1"
        try:
            return bass_utils.run_bass_kernel_spmd(
                nc, in_maps, core_ids=list(range(N_CORES))
            )
        finally:
            os.environ.pop("BASS_NEVER_TRACE", None)


def kernel(hidden_states, router_w, gate_w, up_w, down_w):
    hs = np.asarray(hidden_states)
    rw = np.asarray(router_w)
    gw = np.asarray(gate_w)
    uw = np.asarray(up_w)
    dw = np.asarray(down_w)

    in_maps, meta = _route_and_shard(hs, rw, gw, uw, dw)
    nc = _get_nc(meta["CA"], meta["CB"])
    res = _run_spmd(nc, in_maps)
    return _combine(res.results, meta)
